# revision 4
# baseline (speedup 1.0000x reference)
"""Contrastive-loss Bass kernel for Trainium2 (8 NeuronCores, data-parallel).

Problem (hardcoded shapes, from the reference):
  outA/outB: [4, 307200, 16] f32; matchA/B: [4, 5000] int; nonMatchA/B: [4, 50000] int
  matchLossSum    = sum_b sum((outA[b][matchA[b]] - outB[b][matchB[b]])**2) / 5000
  nonMatchLossSum = sum_b sum(relu(0.5 - (outA[b][nonMatchA[b]] - outB[b][nonMatchB[b]])**2)) / 50000
  returns (contrastiveLossSum, matchLossSum, nonMatchLossSum)

Sharding (per the data-parallel hint): core c handles batch b=c//2 and half
h=c%2 of that batch's match/nonmatch sample lists. Each core indirect-DMA
gathers its rows (one 128-row vector-indirect DMA per index column — the HW
DGE consumes exactly one offset per destination partition), reduces on the
vector engine to per-partition partial sums, and the host does the final tiny
cross-core reduction (equivalent to the all-reduce of three scalars).

Nonmatch hinge is computed as sum(relu(M - d^2)) = M*K - sum(min(d^2, M)),
done with a single fused tensor_scalar(min)+accumulate pass per chunk.

Padding: index lists are padded to multiples of 128 with indices pointing at
two rows appended to each [N, D] tensor:
  row N   = zeros  (match pads: (0-0)^2 = 0 contribution)
  row N+1 = BIG    (nonmatch pads A-side: min(BIG^2, M) = M, which cancels
                    exactly in the M*K - sum(min) identity)
"""

import numpy as np

import concourse.bacc as bacc
import concourse.mybir as mybir
import concourse.tile as tile
from concourse.bass import IndirectOffsetOnAxis

B, N, D = 4, 307200, 16
M, MN = 5000, 50000
NCORES = 8
MARGIN = 0.5
NON_MATCH_W = 1.0
BIG = 1.0e3
NPAD = N + 2          # row N: zeros, row N+1: BIG
M_HALF, MN_HALF = M // 2, MN // 2          # 2500 / 25000 per core
M_COLS = 20           # 128*20  = 2560  match slots  (60 pads)
NM_COLS = 196         # 128*196 = 25088 nonmatch slots (88 pads)
NM_CHUNKS = 4
NM_CCOLS = NM_COLS // NM_CHUNKS            # 49 index cols per chunk
OUT_COLS = NM_CHUNKS + 1                   # per-partition partial sums

_F32 = mybir.dt.float32
_I32 = mybir.dt.int32

_nc_cache = None


def _build():
    nc = bacc.Bacc("TRN2", target_bir_lowering=False, debug=False, num_devices=NCORES)
    A = nc.dram_tensor("A", [NPAD, D], _F32, kind="ExternalInput")
    Bv = nc.dram_tensor("Bv", [NPAD, D], _F32, kind="ExternalInput")
    miA = nc.dram_tensor("miA", [128, M_COLS], _I32, kind="ExternalInput")
    miB = nc.dram_tensor("miB", [128, M_COLS], _I32, kind="ExternalInput")
    niA = nc.dram_tensor("niA", [128, NM_COLS], _I32, kind="ExternalInput")
    niB = nc.dram_tensor("niB", [128, NM_COLS], _I32, kind="ExternalInput")
    out = nc.dram_tensor("out", [128, OUT_COLS], _F32, kind="ExternalOutput")

    with tile.TileContext(nc) as tc:
        with (
            tc.tile_pool(name="idx", bufs=1) as idxp,
            tc.tile_pool(name="gat", bufs=2) as gatp,
            tc.tile_pool(name="tmp", bufs=2) as tmpp,
            tc.tile_pool(name="res", bufs=1) as resp,
        ):
            niA_t = idxp.tile([128, NM_COLS], _I32, tag="ia")
            niB_t = idxp.tile([128, NM_COLS], _I32, tag="ib")
            miA_t = idxp.tile([128, M_COLS], _I32, tag="ma")
            miB_t = idxp.tile([128, M_COLS], _I32, tag="mb")
            nc.sync.dma_start(out=niA_t[:], in_=niA[:])
            nc.sync.dma_start(out=niB_t[:], in_=niB[:])
            nc.sync.dma_start(out=miA_t[:], in_=miA[:])
            nc.sync.dma_start(out=miB_t[:], in_=miB[:])

            res_t = resp.tile([128, OUT_COLS], _F32)

            # nonmatch: res[:, c] = sum_free min((a-b)^2, MARGIN), chunked so
            # gather tiles double-buffer and the SWDGE ring never overfills.
            W = NM_CCOLS * D
            for c in range(NM_CHUNKS):
                ga = gatp.tile([128, W], _F32, tag="ga")
                gb = gatp.tile([128, W], _F32, tag="gb")
                for j in range(NM_CCOLS):
                    col = c * NM_CCOLS + j
                    nc.gpsimd.indirect_dma_start(
                        out=ga[:, j * D : (j + 1) * D], out_offset=None, in_=A[:],
                        in_offset=IndirectOffsetOnAxis(ap=niA_t[:, col : col + 1], axis=0),
                    )
                    nc.gpsimd.indirect_dma_start(
                        out=gb[:, j * D : (j + 1) * D], out_offset=None, in_=Bv[:],
                        in_offset=IndirectOffsetOnAxis(ap=niB_t[:, col : col + 1], axis=0),
                    )
                d_t = tmpp.tile([128, W], _F32, tag="d")
                nc.vector.tensor_tensor(
                    out=d_t[:], in0=ga[:], in1=gb[:], op=mybir.AluOpType.subtract
                )
                sq_t = tmpp.tile([128, W], _F32, tag="sq")
                nc.vector.tensor_tensor(
                    out=sq_t[:], in0=d_t[:], in1=d_t[:], op=mybir.AluOpType.mult
                )
                junk_t = tmpp.tile([128, W], _F32, tag="junk")
                nc.vector.tensor_scalar(
                    out=junk_t[:], in0=sq_t[:],
                    scalar1=MARGIN, scalar2=None, op0=mybir.AluOpType.min,
                    op1=mybir.AluOpType.add,
                    accum_out=res_t[:, c : c + 1],
                )

            # match: res[:, NM_CHUNKS] = sum_free (a-b)^2
            WM = M_COLS * D
            mga = gatp.tile([128, WM], _F32, tag="mga")
            mgb = gatp.tile([128, WM], _F32, tag="mgb")
            for j in range(M_COLS):
                nc.gpsimd.indirect_dma_start(
                    out=mga[:, j * D : (j + 1) * D], out_offset=None, in_=A[:],
                    in_offset=IndirectOffsetOnAxis(ap=miA_t[:, j : j + 1], axis=0),
                )
                nc.gpsimd.indirect_dma_start(
                    out=mgb[:, j * D : (j + 1) * D], out_offset=None, in_=Bv[:],
                    in_offset=IndirectOffsetOnAxis(ap=miB_t[:, j : j + 1], axis=0),
                )
            md_t = tmpp.tile([128, WM], _F32, tag="md")
            nc.vector.tensor_tensor(
                out=md_t[:], in0=mga[:], in1=mgb[:], op=mybir.AluOpType.subtract
            )
            msq_t = tmpp.tile([128, WM], _F32, tag="msq")
            nc.vector.scalar_tensor_tensor(
                out=msq_t[:], in0=md_t[:], scalar=0.0, in1=md_t[:],
                op0=mybir.AluOpType.add, op1=mybir.AluOpType.mult,
                accum_out=res_t[:, NM_CHUNKS : NM_CHUNKS + 1],
            )

            nc.sync.dma_start(out=out[:], in_=res_t[:])
    nc.compile()
    return nc


def _get_nc():
    global _nc_cache
    if _nc_cache is None:
        _nc_cache = _build()
    return _nc_cache


def _pack_idx(idx, ncols, pad_value):
    flat = np.full(128 * ncols, pad_value, dtype=np.int32)
    flat[: idx.size] = idx.astype(np.int32, copy=False)
    return flat.reshape(128, ncols)


def _make_in_maps(outA, outB, matchA, matchB, nonMatchA, nonMatchB):
    pad_zero = np.zeros((1, D), np.float32)
    pad_big = np.full((1, D), BIG, np.float32)
    in_maps = []
    for c in range(NCORES):
        b, h = divmod(c, 2)
        msl = slice(h * M_HALF, (h + 1) * M_HALF)
        nsl = slice(h * MN_HALF, (h + 1) * MN_HALF)
        in_maps.append(
            {
                "A": np.ascontiguousarray(
                    np.concatenate([outA[b], pad_zero, pad_big], axis=0)
                ),
                "Bv": np.ascontiguousarray(
                    np.concatenate([outB[b], pad_zero, pad_zero], axis=0)
                ),
                # match pads -> (N, N): zero rows both sides, zero contribution
                "miA": _pack_idx(matchA[b, msl], M_COLS, N),
                "miB": _pack_idx(matchB[b, msl], M_COLS, N),
                # nonmatch pads -> (N+1, N): d = BIG, min(d^2, MARGIN) = MARGIN cancels
                "niA": _pack_idx(nonMatchA[b, nsl], NM_COLS, N + 1),
                "niB": _pack_idx(nonMatchB[b, nsl], NM_COLS, N),
            }
        )
    return in_maps


def _reduce_results(results):
    m_sum = 0.0
    nm_clip_sum = 0.0
    for c in range(NCORES):
        res = np.asarray(results[c]["out"], dtype=np.float64)
        nm_clip_sum += res[:, :NM_CHUNKS].sum()
        m_sum += res[:, NM_CHUNKS].sum()
    # pads contribute exactly MARGIN per element to the clip sum; the identity
    # below cancels them: sum(relu(M - d^2)) = M*K_slots - sum(min(d^2, M))
    hinge_sum = MARGIN * (128 * NM_COLS * D) * NCORES - nm_clip_sum
    matchLossSum = np.float32(m_sum / M)
    nonMatchLossSum = np.float32(NON_MATCH_W * hinge_sum / MN)
    contrastiveLossSum = np.float32(matchLossSum + nonMatchLossSum)
    return (contrastiveLossSum, matchLossSum, nonMatchLossSum)


def run(inputs, trace=False):
    """Run on the 8 NeuronCores. Returns (result_tuple, exec_time_ns_or_None)."""
    from concourse.bass_utils import run_bass_kernel_spmd

    outA = np.asarray(inputs["outA"], dtype=np.float32)
    outB = np.asarray(inputs["outB"], dtype=np.float32)
    matchA = np.asarray(inputs["matchA"])
    matchB = np.asarray(inputs["matchB"])
    nonMatchA = np.asarray(inputs["nonMatchA"])
    nonMatchB = np.asarray(inputs["nonMatchB"])

    in_maps = _make_in_maps(outA, outB, matchA, matchB, nonMatchA, nonMatchB)
    nc = _get_nc()
    r = run_bass_kernel_spmd(nc, in_maps, list(range(NCORES)), trace=trace)
    out = _reduce_results(r.results)
    ns = r.exec_time_ns
    if ns is None and r.mean_exec_time_ns is not None:
        ns = int(r.mean_exec_time_ns)
    return out, ns


def kernel(**inputs):
    result, _ = run(inputs, trace=False)
    return result
